# revision 10
# baseline (speedup 1.0000x reference)
"""CRNN (conv3x3 -> ReLU -> freq-maxpool -> GRU scan -> FC) on 8 Trainium2
NeuronCores, data-parallel over batch (8 items per core).

Structure per core (v2):
  - conv in fp16: banded-weight matmuls over the frequency contraction; time
    shifts via column offsets into a padded fp16 tile; two accumulating
    matmuls per f-pair give PSUM [128 = 2f x 64c, w]; running tensor_max over
    f-pairs + ReLU(+bias) writes feat[c, t] batch-interleaved into
    bigU[64:128]. Time axis is processed in 256-col chunks: chunk 0 upfront,
    chunks 1-3 interleaved into the scan.
  - xn = W_ihn @ feat precomputed (PE) into bigH[64:128]; b_ihn is folded
    into the tanh bias.
  - GRU scan, 10 instructions/step: ONE sigmoid covers z and r (gate order
    z|r in psum rows), and v_neg = (z-1)*n replaces the separate (1-z)
    sigmoid; h_{k+1} = u_k - v_neg_k with u_k = z_k*h_k. The rz matmul takes
    [u; feat] (K=128) early plus a late v_neg matmul with negated weights, so
    the only late operand on the serial chain is v_neg.
  - FC tiles write de-interleaved into an SBUF outT tile; one contiguous DMA
    per batch at the end.
  - conv chunks 1-3, xn tiles, and FC tiles are emitted interleaved with the
    scan steps so they execute in the scan's idle engine slots.
"""

import contextlib
import numpy as np

import concourse.bass as bass
import concourse.mybir as mybir
import concourse.tile as tile
from concourse import bacc
from concourse.bass_utils import run_bass_kernel_spmd

F32 = mybir.dt.float32
F16 = mybir.dt.float16
AF = mybir.ActivationFunctionType
OP = mybir.AluOpType

B, F, T = 64, 64, 1024
C = 64
H = 64
OUT = 2
NCORES = 8
NB = B // NCORES
NFP = F // 2


def build_crnn(nb=NB, t_steps=T, reps=1, phases=("conv", "xn", "scan", "fc"),
               interleave=True):
    nc = bacc.Bacc("TRN2", target_bir_lowering=False, debug=False)
    TB = t_steps * nb
    NJ = max(1, TB // 512)
    JW = min(512, TB)
    full = len(phases) == 4
    inter = interleave and full and t_steps == T

    x_d = nc.declare_dram_parameter("x", [nb, F, t_steps], F16, isOutput=False)
    convA_d = nc.declare_dram_parameter("convA", [128, NFP * 128], F16, isOutput=False)
    convB_d = nc.declare_dram_parameter("convB", [64, NFP * 128], F16, isOutput=False)
    cb_d = nc.declare_dram_parameter("conv_bias", [C, 1], F32, isOutput=False)
    wrz_d = nc.declare_dram_parameter("w_rz_lhsT", [128, 128], F32, isOutput=False)
    wrzn_d = nc.declare_dram_parameter("w_rz_neg_lhsT", [H, 128], F32, isOutput=False)
    wn_d = nc.declare_dram_parameter("w_n_lhsT", [H, H], F32, isOutput=False)
    win_d = nc.declare_dram_parameter("w_in_lhsT", [C, H], F32, isOutput=False)
    brz_d = nc.declare_dram_parameter("b_rz", [128, 1], F32, isOutput=False)
    bhn_d = nc.declare_dram_parameter("b_hn", [H, 1], F32, isOutput=False)
    bin_d = nc.declare_dram_parameter("b_in_col", [H, 1], F32, isOutput=False)
    fcw_d = nc.declare_dram_parameter("fc_lhsT", [H, OUT], F32, isOutput=False)
    fcb_d = nc.declare_dram_parameter("fc_b_row", [1, OUT], F32, isOutput=False)
    out_d = nc.declare_dram_parameter("out", [nb, OUT, t_steps], F32, isOutput=True)

    with tile.TileContext(nc) as tc:
        with (
            tc.tile_pool(name="persist", bufs=1) as persist,
            tc.tile_pool(name="work", bufs=2) as work,
            tc.tile_pool(name="scanw", bufs=3) as scanw,
            tc.tile_pool(name="pp_conv", bufs=2, space="PSUM") as ppc,
            tc.tile_pool(name="pp_scan", bufs=2, space="PSUM") as pps,
            tc.tile_pool(name="pp_misc", bufs=2, space="PSUM") as ppm,
        ):
            convA = persist.tile([128, NFP * 128], F16)
            convB = persist.tile([64, NFP * 128], F16)
            cb = persist.tile([C, 1], F32)
            w_rz = persist.tile([128, 128], F32)
            w_rz_neg = persist.tile([H, 128], F32)
            w_n = persist.tile([H, H], F32)
            w_in_full = persist.tile([128, H], F32)
            w_in = w_in_full[64:128, :]
            b_rz = persist.tile([128, 1], F32)
            b_hn_full = persist.tile([128, 1], F32)
            b_hn = b_hn_full[64:128, :]
            b_in_full = persist.tile([128, 1], F32)
            b_in = b_in_full[64:128, :]
            fc_w = persist.tile([H, OUT], F32)
            fc_b = persist.tile([1, OUT], F32)
            ones = persist.tile([1, JW], F32)
            # bigU: rows 0:64 = u_{k-1} at blk k, rows 64:128 = feat_k at blk k
            bigU = persist.tile([128, (t_steps + 1) * nb], F32)
            # bigH: rows 0:64 = h_k at blk k, rows 64:128 = xn_k at blk k
            bigH = persist.tile([128, (t_steps + 1) * nb], F32)
            obBs = [persist.tile([OUT, t_steps], F32, name=f"ob{b}")
                    for b in range(nb)]
            v_zero = persist.tile([H, nb], F32)

            nc.sync.dma_start(out=convA, in_=convA_d[:, :])
            nc.sync.dma_start(out=convB, in_=convB_d[:, :])
            nc.sync.dma_start(out=cb, in_=cb_d[:, :])
            nc.sync.dma_start(out=w_rz, in_=wrz_d[:, :])
            nc.sync.dma_start(out=w_rz_neg, in_=wrzn_d[:, :])
            nc.sync.dma_start(out=w_n, in_=wn_d[:, :])
            nc.sync.dma_start(out=w_in, in_=win_d[:, :])
            nc.sync.dma_start(out=b_rz, in_=brz_d[:, :])
            nc.sync.dma_start(out=b_hn, in_=bhn_d[:, :])
            nc.sync.dma_start(out=b_in, in_=bin_d[:, :])
            nc.sync.dma_start(out=fc_w, in_=fcw_d[:, :])
            nc.sync.dma_start(out=fc_b, in_=fcb_d[:, :])
            nc.vector.memset(ones, 1.0)
            nc.vector.memset(bigU[0:64, 0:nb], 0.0)   # u_{-1} = 0
            nc.vector.memset(bigH[0:64, 0:nb], 0.0)   # h_0 = 0
            nc.vector.memset(v_zero, 0.0)             # v_neg_{-1} = 0
            if not full:
                nc.vector.memset(bigU[:, :], 0.0)
                nc.vector.memset(bigH[:, :], 0.0)

            # ---------- X2 staging (persistent fp16, per batch) ----------
            X2s = []
            if "conv" in phases:
                for b in range(nb):
                    X2 = persist.tile([128, t_steps + 2], F16, name=f"x2_{b}")
                    nc.sync.dma_start(out=X2[0:64, 1 : t_steps + 1], in_=x_d[b, :, :])
                    nc.sync.dma_start(out=X2[64:128, 0:t_steps], in_=x_d[b, :, :])
                    nc.vector.memset(X2[0:64, 0:1], 0.0)
                    nc.vector.memset(X2[0:64, t_steps + 1 : t_steps + 2], 0.0)
                    nc.vector.memset(X2[64:128, t_steps : t_steps + 2], 0.0)
                    X2s.append(X2)

            # ---------- emission units ----------
            conv_state = {}

            def conv_mm(b, s, w, fp):
                # conv output columns t in [s, s+w)
                ps = ppc.tile([128, w], F32, tag="cps", name="cps")
                X2 = X2s[b]
                nc.tensor.matmul(
                    ps, convA[:, fp * 128 : (fp + 1) * 128],
                    X2[:, s : s + w], start=True, stop=False,
                )
                nc.tensor.matmul(
                    ps, convB[:, fp * 128 : (fp + 1) * 128],
                    X2[0:64, s + 2 : s + w + 2], start=False, stop=True,
                )
                if fp == 0:
                    macc = work.tile([128, w], F32, tag="macc", name="macc")
                    conv_state[(b, s)] = macc
                    nc.vector.tensor_copy(macc, ps)
                else:
                    nc.vector.tensor_max(conv_state[(b, s)],
                                         conv_state[(b, s)], ps)

            def conv_tail(b, s, w):
                macc = conv_state.pop((b, s))
                mhi = work.tile([64, w], F32, tag="mhi", name="mhi")
                nc.vector.tensor_copy(mhi, macc[64:128, :])
                m2 = work.tile([64, w], F32, tag="m2", name="m2")
                nc.vector.tensor_max(m2, macc[0:64, :], mhi)
                out_ap = bigU[64:128, s * nb + b : (s + w) * nb : nb]
                nc.scalar.activation(out_ap, m2, AF.Relu, bias=cb)

            def xn_unit(j):
                ps = ppm.tile([H, JW], F32, tag="mps", name="xnps")
                nc.tensor.matmul(
                    ps, w_in, bigU[64:128, j * JW : (j + 1) * JW],
                    start=True, stop=True,
                )
                nc.scalar.copy(bigH[64:128, j * JW : (j + 1) * JW], ps)

            FCW = min(512, t_steps)

            def fc_unit(b, half):
                # output t range [half*FCW, (half+1)*FCW) for batch b
                base = nb + b + half * FCW * nb
                ps = ppm.tile([OUT, FCW], F32, tag="mps", name="fcps")
                nc.tensor.matmul(
                    ps, fc_w, bigH[0:64, base : base + (FCW - 1) * nb + 1 : nb],
                    start=True, stop=False,
                )
                nc.tensor.matmul(ps, fc_b, ones[:, 0:FCW], start=False, stop=True)
                nc.scalar.copy(obBs[b][:, half * FCW : (half + 1) * FCW], ps)

            def scan_step(k, prev_vn):
                col = slice(k * nb, (k + 1) * nb)
                ncol = slice((k + 1) * nb, (k + 2) * nb)
                # psum_rz rows: 0:64 z-pre, 64:128 r-pre (gate order z|r)
                psum_rz = pps.tile([128, nb], F32, tag="rz", name="rz")
                psum_hn = pps.tile([128, nb], F32, tag="hn", name="hn")
                nc.tensor.matmul(psum_rz, w_rz, bigU[:, col], start=True, stop=False)
                nc.tensor.matmul(psum_hn[64:128, :], w_n, bigH[0:64, col],
                                 start=True, stop=True)
                nc.tensor.matmul(psum_rz, w_rz_neg, prev_vn, start=False, stop=True)

                sig = scanw.tile([128, nb], F32, tag="sig", name="sig")
                nc.scalar.activation(sig, psum_rz, AF.Sigmoid, bias=b_rz)
                # q = (hn_pre + b_hn) * r     (rows 64:128)
                q = scanw.tile([128, nb], F32, tag="q", name="q")
                nc.vector.scalar_tensor_tensor(
                    out=q[64:128, :], in0=psum_hn[64:128, :], scalar=b_hn,
                    in1=sig[64:128, :], op0=OP.add, op1=OP.mult,
                )
                q2 = scanw.tile([128, nb], F32, tag="q2", name="q2")
                nc.vector.tensor_add(q2[64:128, :], q[64:128, :], bigH[64:128, col])
                # u_k = z_k * h_k
                nc.vector.tensor_mul(bigU[0:64, ncol], sig[0:64, :], bigH[0:64, col])
                n_t = scanw.tile([H, nb], F32, tag="n", name="n")
                nc.scalar.activation(n_t, q2[64:128, :], AF.Tanh, bias=b_in)
                # v_neg = (z - 1) * n
                vn = scanw.tile([H, nb], F32, tag="v", name="v")
                nc.vector.scalar_tensor_tensor(
                    out=vn, in0=sig[0:64, :], scalar=-1.0, in1=n_t,
                    op0=OP.add, op1=OP.mult,
                )
                # h_{k+1} = u_k - v_neg
                nc.vector.tensor_sub(bigH[0:64, ncol], bigU[0:64, ncol], vn)
                return vn

            # conv chunk plan: list of (start, width)
            CW = 256
            chunks = [(s, min(CW, t_steps - s)) for s in range(0, t_steps, CW)]

            def emit_conv_chunk(s, w):
                for b in range(nb):
                    for fp in range(NFP):
                        conv_mm(b, s, w, fp)
                    conv_tail(b, s, w)

            rep_ctx = tc.For_i(0, reps, 1) if reps > 1 else contextlib.nullcontext()
            with rep_ctx:
                if not inter:
                    if "conv" in phases:
                        for s, w in chunks:
                            emit_conv_chunk(s, w)
                    for j in range(NJ if "xn" in phases else 0):
                        xn_unit(j)
                    prev_vn = v_zero
                    for k in range(t_steps if "scan" in phases else 0):
                        prev_vn = scan_step(k, prev_vn)
                    if "fc" in phases:
                        for half in range(max(1, t_steps // FCW)):
                            for b in range(nb):
                                fc_unit(b, half)
                else:
                    # upfront: conv chunk 0 (t in [0,256)) + xn tiles j=0..3
                    emit_conv_chunk(*chunks[0])
                    for j in range(4):
                        xn_unit(j)

                    # interleave plan: step -> list of thunks
                    sched = {}

                    def spread(units, lo, hi):
                        n = len(units)
                        for i, u in enumerate(units):
                            k_at = lo + (i * (hi - lo)) // n
                            sched.setdefault(k_at, []).append(u)

                    # conv chunk 1 (t in [256,512)): emit over steps [4,190)
                    # then its xn tiles j=4..7 at steps [195,227)
                    # conv chunk 2 over [255,440), xn j=8..11 at [445,477)
                    # conv chunk 3 over [485,700), xn j=12..15 at [705,737)
                    def conv_units(s, w):
                        us = []
                        for b in range(nb):
                            for fp in range(NFP):
                                us.append(lambda b=b, fp=fp: conv_mm(b, s, w, fp))
                            us.append(lambda b=b: conv_tail(b, s, w))
                        return us

                    spread(conv_units(*chunks[1]), 4, 190)
                    spread([lambda j=j: xn_unit(j) for j in range(4, 8)], 195, 227)
                    spread(conv_units(*chunks[2]), 255, 440)
                    spread([lambda j=j: xn_unit(j) for j in range(8, 12)], 445, 477)
                    spread(conv_units(*chunks[3]), 485, 700)
                    spread([lambda j=j: xn_unit(j) for j in range(12, 16)], 705, 737)
                    fc_tail = []
                    for half in range(t_steps // FCW):
                        for b in range(nb):
                            k_at = (half + 1) * FCW + 2 + 6 * b
                            if k_at < t_steps:
                                sched.setdefault(k_at, []).append(
                                    lambda b=b, half=half: fc_unit(b, half))
                            else:
                                fc_tail.append((b, half))

                    prev_vn = v_zero
                    for k in range(t_steps):
                        prev_vn = scan_step(k, prev_vn)
                        for u in sched.get(k, ()):
                            u()
                    for b, half in fc_tail:
                        fc_unit(b, half)

                if "fc" in phases:
                    for b in range(nb):
                        nc.sync.dma_start(out=out_d[b, :, :], in_=obBs[b])

    nc.finalize()
    return nc


def prep_weights(conv_w, conv_b, w_ih, w_hh, b_ih, b_hh, fc_w, fc_b):
    """Host-side rearrangement of the small weights into device layouts."""
    conv_w = np.asarray(conv_w, np.float32)
    A = np.zeros((128, NFP * 128), np.float32)
    Bm = np.zeros((64, NFP * 128), np.float32)
    for fp in range(NFP):
        for fo in range(2):
            fout = 2 * fp + fo
            for fprime in range(max(0, fout - 1), min(64, fout + 2)):
                i = fprime - fout + 1
                cols = slice(fp * 128 + fo * 64, fp * 128 + fo * 64 + 64)
                A[fprime, cols] = conv_w[:, 0, i, 0]
                A[64 + fprime, cols] = conv_w[:, 0, i, 1]
                Bm[fprime, cols] = conv_w[:, 0, i, 2]
    w_ih = np.asarray(w_ih, np.float32)
    w_hh = np.asarray(w_hh, np.float32)
    b_ih = np.asarray(b_ih, np.float32)
    b_hh = np.asarray(b_hh, np.float32)
    zr = np.r_[64:128, 0:64]        # gate order z|r
    w_rz = np.concatenate([w_hh[0:128][zr].T, w_ih[0:128][zr].T], axis=0)
    return {
        "convA": A.astype(np.float16),
        "convB": Bm.astype(np.float16),
        "conv_bias": np.asarray(conv_b, np.float32).reshape(C, 1),
        "w_rz_lhsT": w_rz.astype(np.float32).copy(),
        "w_rz_neg_lhsT": (-w_hh[0:128][zr].T).astype(np.float32).copy(),
        "w_n_lhsT": w_hh[128:192, :].T.astype(np.float32).copy(),
        "w_in_lhsT": w_ih[128:192, :].T.astype(np.float32).copy(),
        "b_rz": (b_ih[0:128] + b_hh[0:128])[zr].reshape(128, 1).astype(np.float32),
        "b_hn": b_hh[128:192].reshape(H, 1).astype(np.float32),
        "b_in_col": b_ih[128:192].reshape(H, 1).astype(np.float32),
        "fc_lhsT": np.asarray(fc_w, np.float32).T.copy(),
        "fc_b_row": np.asarray(fc_b, np.float32).reshape(1, OUT),
    }


def make_in_maps(inputs):
    x = np.asarray(inputs["x"], np.float32)
    wd = prep_weights(
        inputs["conv_w"], inputs["conv_b"], inputs["w_ih"], inputs["w_hh"],
        inputs["b_ih"], inputs["b_hh"], inputs["fc_w"], inputs["fc_b"],
    )
    in_maps = []
    for i in range(NCORES):
        m = dict(wd)
        m["x"] = np.ascontiguousarray(x[i * NB : (i + 1) * NB]).astype(np.float16)
        in_maps.append(m)
    return in_maps


_NC_CACHE = {}


def _get_nc():
    if "nc" not in _NC_CACHE:
        _NC_CACHE["nc"] = build_crnn()
    return _NC_CACHE["nc"]


def run(inputs, trace=False):
    """Returns (out [B, OUT, T], BassKernelResults)."""
    nc = _get_nc()
    in_maps = make_in_maps(inputs)
    res = run_bass_kernel_spmd(nc, in_maps, list(range(NCORES)), trace=trace)
    out = np.concatenate([res.results[i]["out"] for i in range(NCORES)], axis=0)
    return out, res


def kernel(**inputs) -> np.ndarray:
    out, _ = run(inputs, trace=False)
    return out


# revision 12
# speedup vs baseline: 1.1963x; 1.1963x over previous
"""CRNN (conv3x3 -> ReLU -> freq-maxpool -> GRU scan -> FC) on 8 Trainium2
NeuronCores, data-parallel over batch (8 items per core).

Structure per core (v2):
  - conv in fp16: banded-weight matmuls over the frequency contraction; time
    shifts via column offsets into a padded fp16 tile; two accumulating
    matmuls per f-pair give PSUM [128 = 2f x 64c, w]; running tensor_max over
    f-pairs + ReLU(+bias) writes feat[c, t] batch-interleaved into
    bigU[64:128]. Time axis is processed in 256-col chunks: chunk 0 upfront,
    chunks 1-3 interleaved into the scan.
  - xn = W_ihn @ feat precomputed (PE) into bigH[64:128]; b_ihn is folded
    into the tanh bias.
  - GRU scan, 10 instructions/step: ONE sigmoid covers z and r (gate order
    z|r in psum rows), and v_neg = (z-1)*n replaces the separate (1-z)
    sigmoid; h_{k+1} = u_k - v_neg_k with u_k = z_k*h_k. The rz matmul takes
    [u; feat] (K=128) early plus a late v_neg matmul with negated weights, so
    the only late operand on the serial chain is v_neg.
  - FC tiles write de-interleaved into an SBUF outT tile; one contiguous DMA
    per batch at the end.
  - conv chunks 1-3, xn tiles, and FC tiles are emitted interleaved with the
    scan steps so they execute in the scan's idle engine slots.
"""

import contextlib
import numpy as np

import concourse.bass as bass
import concourse.mybir as mybir
import concourse.tile as tile
from concourse import bacc
from concourse.bass_utils import run_bass_kernel_spmd

F32 = mybir.dt.float32
F16 = mybir.dt.float16
AF = mybir.ActivationFunctionType
OP = mybir.AluOpType

B, F, T = 64, 64, 1024
C = 64
H = 64
OUT = 2
NCORES = 8
NB = B // NCORES
NFP = F // 2


def build_crnn(nb=NB, t_steps=T, reps=1, phases=("conv", "xn", "scan", "fc"),
               interleave=True):
    nc = bacc.Bacc("TRN2", target_bir_lowering=False, debug=False)
    TB = t_steps * nb
    NJ = max(1, TB // 512)
    JW = min(512, TB)
    full = len(phases) == 4
    inter = interleave and full and t_steps == T

    x_d = nc.declare_dram_parameter("x", [nb, F, t_steps], F16, isOutput=False)
    convA_d = nc.declare_dram_parameter("convA", [128, NFP * 128], F16, isOutput=False)
    convB_d = nc.declare_dram_parameter("convB", [64, NFP * 128], F16, isOutput=False)
    cb_d = nc.declare_dram_parameter("conv_bias", [C, 1], F32, isOutput=False)
    wrz_d = nc.declare_dram_parameter("w_rz_lhsT", [128, 128], F32, isOutput=False)
    wrzn_d = nc.declare_dram_parameter("w_rz_neg_lhsT", [H, 128], F32, isOutput=False)
    wn_d = nc.declare_dram_parameter("w_n_lhsT", [H, H], F32, isOutput=False)
    win_d = nc.declare_dram_parameter("w_in_lhsT", [C, H], F32, isOutput=False)
    brz_d = nc.declare_dram_parameter("b_rz", [128, 1], F32, isOutput=False)
    bhn_d = nc.declare_dram_parameter("b_hn", [H, 1], F32, isOutput=False)
    bin_d = nc.declare_dram_parameter("b_in_col", [H, 1], F32, isOutput=False)
    fcw_d = nc.declare_dram_parameter("fc_lhsT", [H, OUT], F32, isOutput=False)
    fcb_d = nc.declare_dram_parameter("fc_b_row", [1, OUT], F32, isOutput=False)
    out_d = nc.declare_dram_parameter("out", [nb, OUT, t_steps], F32, isOutput=True)

    with tile.TileContext(nc) as tc:
        with (
            tc.tile_pool(name="persist", bufs=1) as persist,
            tc.tile_pool(name="work", bufs=2) as work,
            tc.tile_pool(name="scanw", bufs=3) as scanw,
            tc.tile_pool(name="pp_conv", bufs=2, space="PSUM") as ppc,
            tc.tile_pool(name="pp_scan", bufs=2, space="PSUM") as pps,
            tc.tile_pool(name="pp_misc", bufs=2, space="PSUM") as ppm,
        ):
            convA = persist.tile([128, NFP * 128], F16)
            convB = persist.tile([64, NFP * 128], F16)
            cb = persist.tile([C, 1], F32)
            w_rz = persist.tile([128, 128], F32)
            w_rz_neg = persist.tile([H, 128], F32)
            w_n = persist.tile([H, H], F32)
            w_in_full = persist.tile([128, H], F32)
            w_in = w_in_full[64:128, :]
            b_rz = persist.tile([128, 1], F32)
            b_hn_full = persist.tile([128, 1], F32)
            b_hn = b_hn_full[64:128, :]
            b_in_full = persist.tile([128, 1], F32)
            b_in = b_in_full[64:128, :]
            fc_w = persist.tile([H, OUT], F32)
            fc_b = persist.tile([1, OUT], F32)
            ones = persist.tile([1, JW], F32)
            # bigU: rows 0:64 = u_{k-1} at blk k, rows 64:128 = feat_k at blk k
            bigU = persist.tile([128, (t_steps + 1) * nb], F32)
            # bigH: rows 0:64 = h_k at blk k, rows 64:128 = xn_k at blk k
            bigH = persist.tile([128, (t_steps + 1) * nb], F32)
            obBs = [persist.tile([OUT, t_steps], F32, name=f"ob{b}")
                    for b in range(nb)]
            v_zero = persist.tile([H, nb], F32)

            nc.sync.dma_start(out=convA, in_=convA_d[:, :])
            nc.sync.dma_start(out=convB, in_=convB_d[:, :])
            nc.sync.dma_start(out=cb, in_=cb_d[:, :])
            nc.sync.dma_start(out=w_rz, in_=wrz_d[:, :])
            nc.sync.dma_start(out=w_rz_neg, in_=wrzn_d[:, :])
            nc.sync.dma_start(out=w_n, in_=wn_d[:, :])
            nc.sync.dma_start(out=w_in, in_=win_d[:, :])
            nc.sync.dma_start(out=b_rz, in_=brz_d[:, :])
            nc.sync.dma_start(out=b_hn, in_=bhn_d[:, :])
            nc.sync.dma_start(out=b_in, in_=bin_d[:, :])
            nc.sync.dma_start(out=fc_w, in_=fcw_d[:, :])
            nc.sync.dma_start(out=fc_b, in_=fcb_d[:, :])
            nc.vector.memset(ones, 1.0)
            nc.vector.memset(bigU[0:64, 0:nb], 0.0)   # u_{-1} = 0
            nc.vector.memset(bigH[0:64, 0:nb], 0.0)   # h_0 = 0
            nc.vector.memset(v_zero, 0.0)             # v_neg_{-1} = 0
            if not full:
                nc.vector.memset(bigU[:, :], 0.0)
                nc.vector.memset(bigH[:, :], 0.0)

            # ---------- X2 staging (persistent fp16, per batch) ----------
            X2s = []
            if "conv" in phases:
                for b in range(nb):
                    X2 = persist.tile([128, t_steps + 2], F16, name=f"x2_{b}")
                    nc.sync.dma_start(out=X2[0:64, 1 : t_steps + 1], in_=x_d[b, :, :])
                    nc.sync.dma_start(out=X2[64:128, 0:t_steps], in_=x_d[b, :, :])
                    nc.vector.memset(X2[0:64, 0:1], 0.0)
                    nc.vector.memset(X2[0:64, t_steps + 1 : t_steps + 2], 0.0)
                    nc.vector.memset(X2[64:128, t_steps : t_steps + 2], 0.0)
                    X2s.append(X2)

            # ---------- emission units ----------
            conv_state = {}

            def conv_mm(b, s, w, fp):
                # conv output columns t in [s, s+w)
                ps = ppc.tile([128, w], F32, tag="cps", name="cps")
                X2 = X2s[b]
                nc.tensor.matmul(
                    ps, convA[:, fp * 128 : (fp + 1) * 128],
                    X2[:, s : s + w], start=True, stop=False,
                )
                nc.tensor.matmul(
                    ps, convB[:, fp * 128 : (fp + 1) * 128],
                    X2[0:64, s + 2 : s + w + 2], start=False, stop=True,
                )
                if fp == 0:
                    macc = work.tile([128, w], F32, tag="macc", name="macc")
                    conv_state[(b, s)] = macc
                    nc.vector.tensor_copy(macc, ps)
                else:
                    nc.vector.tensor_max(conv_state[(b, s)],
                                         conv_state[(b, s)], ps)

            def conv_tail(b, s, w):
                macc = conv_state.pop((b, s))
                mhi = work.tile([64, w], F32, tag="mhi", name="mhi")
                nc.vector.tensor_copy(mhi, macc[64:128, :])
                m2 = work.tile([64, w], F32, tag="m2", name="m2")
                nc.vector.tensor_max(m2, macc[0:64, :], mhi)
                out_ap = bigU[64:128, s * nb + b : (s + w) * nb : nb]
                nc.scalar.activation(out_ap, m2, AF.Relu, bias=cb)

            def xn_unit(j):
                ps = ppm.tile([H, JW], F32, tag="mps", name="xnps")
                nc.tensor.matmul(
                    ps, w_in, bigU[64:128, j * JW : (j + 1) * JW],
                    start=True, stop=True,
                )
                nc.scalar.copy(bigH[64:128, j * JW : (j + 1) * JW], ps)

            FCW = min(512, t_steps)

            def fc_unit(b, half):
                # output t range [half*FCW, (half+1)*FCW) for batch b
                base = nb + b + half * FCW * nb
                ps = ppm.tile([OUT, FCW], F32, tag="mps", name="fcps")
                nc.tensor.matmul(
                    ps, fc_w, bigH[0:64, base : base + (FCW - 1) * nb + 1 : nb],
                    start=True, stop=False,
                )
                nc.tensor.matmul(ps, fc_b, ones[:, 0:FCW], start=False, stop=True)
                nc.scalar.copy(obBs[b][:, half * FCW : (half + 1) * FCW], ps)

            def scan_step(k, prev_vn, pres=()):
                col = slice(k * nb, (k + 1) * nb)
                ncol = slice((k + 1) * nb, (k + 2) * nb)
                # psum_rz rows: 0:64 z-pre, 64:128 r-pre (gate order z|r)
                psum_rz = pps.tile([128, nb], F32, tag="rz", name="rz")
                psum_hn = pps.tile([128, nb], F32, tag="hn", name="hn")
                nc.tensor.matmul(psum_rz, w_rz, bigU[:, col], start=True, stop=False)
                nc.tensor.matmul(psum_hn[64:128, :], w_n, bigH[0:64, col],
                                 start=True, stop=True)
                # interleaved PE/DVE work lands here: it executes inside the
                # wait-for-vn window instead of delaying the critical m2.
                for p in pres:
                    p()
                nc.tensor.matmul(psum_rz, w_rz_neg, prev_vn, start=False, stop=True)

                sig = scanw.tile([128, nb], F32, tag="sig", name="sig")
                nc.scalar.activation(sig, psum_rz, AF.Sigmoid, bias=b_rz)
                # q = (hn_pre + b_hn) * r     (rows 64:128)
                q = scanw.tile([128, nb], F32, tag="q", name="q")
                nc.vector.scalar_tensor_tensor(
                    out=q[64:128, :], in0=psum_hn[64:128, :], scalar=b_hn,
                    in1=sig[64:128, :], op0=OP.add, op1=OP.mult,
                )
                q2 = scanw.tile([128, nb], F32, tag="q2", name="q2")
                nc.vector.tensor_add(q2[64:128, :], q[64:128, :], bigH[64:128, col])
                # u_k = z_k * h_k
                nc.vector.tensor_mul(bigU[0:64, ncol], sig[0:64, :], bigH[0:64, col])
                n_t = scanw.tile([H, nb], F32, tag="n", name="n")
                nc.scalar.activation(n_t, q2[64:128, :], AF.Tanh, bias=b_in)
                # v_neg = (z - 1) * n
                vn = scanw.tile([H, nb], F32, tag="v", name="v")
                nc.vector.scalar_tensor_tensor(
                    out=vn, in0=sig[0:64, :], scalar=-1.0, in1=n_t,
                    op0=OP.add, op1=OP.mult,
                )
                # h_{k+1} = u_k - v_neg
                nc.vector.tensor_sub(bigH[0:64, ncol], bigU[0:64, ncol], vn)
                return vn

            # conv chunk plan: list of (start, width)
            CW = 256
            chunks = [(s, min(CW, t_steps - s)) for s in range(0, t_steps, CW)]

            def emit_conv_chunk(s, w):
                for b in range(nb):
                    for fp in range(NFP):
                        conv_mm(b, s, w, fp)
                    conv_tail(b, s, w)

            rep_ctx = tc.For_i(0, reps, 1) if reps > 1 else contextlib.nullcontext()
            with rep_ctx:
                if not inter:
                    if "conv" in phases:
                        for s, w in chunks:
                            emit_conv_chunk(s, w)
                    for j in range(NJ if "xn" in phases else 0):
                        xn_unit(j)
                    prev_vn = v_zero
                    for k in range(t_steps if "scan" in phases else 0):
                        prev_vn = scan_step(k, prev_vn)
                    if "fc" in phases:
                        for half in range(max(1, t_steps // FCW)):
                            for b in range(nb):
                                fc_unit(b, half)
                else:
                    # upfront: conv chunk 0 (t in [0,256)) + xn tiles j=0..3
                    emit_conv_chunk(*chunks[0])
                    for j in range(4):
                        xn_unit(j)

                    # interleave plan: step -> ([pre thunks], [post thunks]).
                    # pre = PE/DVE work emitted inside scan_step before m2;
                    # post = ACT-containing work emitted after the step.
                    sched_pre = {}
                    sched_post = {}

                    def spread(units, lo, hi):
                        n = len(units)
                        for i, (pre, post) in enumerate(units):
                            k_at = lo + (i * (hi - lo)) // n
                            if pre is not None:
                                sched_pre.setdefault(k_at, []).append(pre)
                            if post is not None:
                                sched_post.setdefault(k_at, []).append(post)

                    def conv_units(s, w):
                        us = []
                        for b in range(nb):
                            for fp in range(NFP):
                                us.append(
                                    (lambda b=b, fp=fp: conv_mm(b, s, w, fp), None))
                            us.append((None, lambda b=b: conv_tail(b, s, w)))
                        return us

                    xn_state = {}

                    def xn_pre(j):
                        ps = ppm.tile([H, JW], F32, tag="mps", name="xnps")
                        nc.tensor.matmul(
                            ps, w_in, bigU[64:128, j * JW : (j + 1) * JW],
                            start=True, stop=True,
                        )
                        xn_state[j] = ps

                    def xn_post(j):
                        nc.scalar.copy(
                            bigH[64:128, j * JW : (j + 1) * JW], xn_state.pop(j))

                    def xn_units(js):
                        return [(lambda j=j: xn_pre(j), lambda j=j: xn_post(j))
                                for j in js]

                    fc_state = {}

                    def fc_pre(b, half):
                        base = nb + b + half * FCW * nb
                        ps = ppm.tile([OUT, FCW], F32, tag="mps", name="fcps")
                        nc.tensor.matmul(
                            ps, fc_w,
                            bigH[0:64, base : base + (FCW - 1) * nb + 1 : nb],
                            start=True, stop=False,
                        )
                        nc.tensor.matmul(ps, fc_b, ones[:, 0:FCW],
                                         start=False, stop=True)
                        fc_state[(b, half)] = ps

                    def fc_post(b, half):
                        nc.scalar.copy(
                            obBs[b][:, half * FCW : (half + 1) * FCW],
                            fc_state.pop((b, half)))

                    # conv chunk 1 (t in [256,512)) over steps [4,140);
                    # xn j=4..7 at [150,240). chunk 2 over [150,400),
                    # xn j=8..11 at [420,500). chunk 3 over [420,740),
                    # xn j=12..15 at [745,765).
                    spread(conv_units(*chunks[1]), 4, 140)
                    spread(xn_units(range(4, 8)), 150, 240)
                    spread(conv_units(*chunks[2]), 150, 400)
                    spread(xn_units(range(8, 12)), 420, 500)
                    spread(conv_units(*chunks[3]), 420, 740)
                    spread(xn_units(range(12, 16)), 745, 765)
                    fc_tail = []
                    for half in range(t_steps // FCW):
                        for b in range(nb):
                            k_at = (half + 1) * FCW + 2 + 6 * b
                            if k_at < t_steps:
                                spread([(lambda b=b, h=half: fc_pre(b, h),
                                         lambda b=b, h=half: fc_post(b, h))],
                                       k_at, k_at + 1)
                            else:
                                fc_tail.append((b, half))

                    prev_vn = v_zero
                    for k in range(t_steps):
                        prev_vn = scan_step(k, prev_vn, sched_pre.get(k, ()))
                        for u in sched_post.get(k, ()):
                            u()
                    for b, half in fc_tail:
                        fc_unit(b, half)

                if "fc" in phases:
                    for b in range(nb):
                        nc.sync.dma_start(out=out_d[b, :, :], in_=obBs[b])

    nc.finalize()
    return nc


def prep_weights(conv_w, conv_b, w_ih, w_hh, b_ih, b_hh, fc_w, fc_b):
    """Host-side rearrangement of the small weights into device layouts."""
    conv_w = np.asarray(conv_w, np.float32)
    A = np.zeros((128, NFP * 128), np.float32)
    Bm = np.zeros((64, NFP * 128), np.float32)
    for fp in range(NFP):
        for fo in range(2):
            fout = 2 * fp + fo
            for fprime in range(max(0, fout - 1), min(64, fout + 2)):
                i = fprime - fout + 1
                cols = slice(fp * 128 + fo * 64, fp * 128 + fo * 64 + 64)
                A[fprime, cols] = conv_w[:, 0, i, 0]
                A[64 + fprime, cols] = conv_w[:, 0, i, 1]
                Bm[fprime, cols] = conv_w[:, 0, i, 2]
    w_ih = np.asarray(w_ih, np.float32)
    w_hh = np.asarray(w_hh, np.float32)
    b_ih = np.asarray(b_ih, np.float32)
    b_hh = np.asarray(b_hh, np.float32)
    zr = np.r_[64:128, 0:64]        # gate order z|r
    w_rz = np.concatenate([w_hh[0:128][zr].T, w_ih[0:128][zr].T], axis=0)
    return {
        "convA": A.astype(np.float16),
        "convB": Bm.astype(np.float16),
        "conv_bias": np.asarray(conv_b, np.float32).reshape(C, 1),
        "w_rz_lhsT": w_rz.astype(np.float32).copy(),
        "w_rz_neg_lhsT": (-w_hh[0:128][zr].T).astype(np.float32).copy(),
        "w_n_lhsT": w_hh[128:192, :].T.astype(np.float32).copy(),
        "w_in_lhsT": w_ih[128:192, :].T.astype(np.float32).copy(),
        "b_rz": (b_ih[0:128] + b_hh[0:128])[zr].reshape(128, 1).astype(np.float32),
        "b_hn": b_hh[128:192].reshape(H, 1).astype(np.float32),
        "b_in_col": b_ih[128:192].reshape(H, 1).astype(np.float32),
        "fc_lhsT": np.asarray(fc_w, np.float32).T.copy(),
        "fc_b_row": np.asarray(fc_b, np.float32).reshape(1, OUT),
    }


def make_in_maps(inputs):
    x = np.asarray(inputs["x"], np.float32)
    wd = prep_weights(
        inputs["conv_w"], inputs["conv_b"], inputs["w_ih"], inputs["w_hh"],
        inputs["b_ih"], inputs["b_hh"], inputs["fc_w"], inputs["fc_b"],
    )
    in_maps = []
    for i in range(NCORES):
        m = dict(wd)
        m["x"] = np.ascontiguousarray(x[i * NB : (i + 1) * NB]).astype(np.float16)
        in_maps.append(m)
    return in_maps


_NC_CACHE = {}


def _get_nc():
    if "nc" not in _NC_CACHE:
        _NC_CACHE["nc"] = build_crnn()
    return _NC_CACHE["nc"]


def run(inputs, trace=False):
    """Returns (out [B, OUT, T], BassKernelResults)."""
    nc = _get_nc()
    in_maps = make_in_maps(inputs)
    res = run_bass_kernel_spmd(nc, in_maps, list(range(NCORES)), trace=trace)
    out = np.concatenate([res.results[i]["out"] for i in range(NCORES)], axis=0)
    return out, res


def kernel(**inputs) -> np.ndarray:
    out, _ = run(inputs, trace=False)
    return out
